# revision 33
# baseline (speedup 1.0000x reference)
"""Multi-head attention (B=4, S=1024, DM=1024, H=16, D=64) on 8 Trainium2 cores.

Sharding: core c handles batch b = c//2 and heads [8*(c%2), 8*(c%2)+8)
(tensor-parallel over heads x data-parallel over batch).

fp32r trick: Trainium2's fp32r matmul dtype is fp32 with the mantissa
rounded to 11 bits (low 12 bits zero) and runs at full bf16 PE rate for
free dims >= 256. The BIR verifier requires fp32r matmul operands to be
*produced* as fp32r, so the host pre-rounds (RNE at bit 12) and
pre-transposes Q/K/V, and the DRAM inputs are declared float32r — DMA
lands matmul-ready tiles with no on-device transposes or conversion
copies.

Per core:
  1. DMA QT/KT/VT [DM, S] (host-pretransposed, fp32r) + W slices (fp32r).
  2. Projections (fp32r matmuls, N=512): QhT/KhT [128hd(head pair), S]
     = W^T X^T; Vh natural [k, hd] with an interleaved ones column per
     head ([Vh | 1] -> AV matmul also produces softmax denominators).
  3. Scores S^T[k, q] = KhT^T @ QhT per head, two heads row-packed in
     the PE array (K=64 at partition offsets 0/64) into one 2-bank PSUM
     tile. Causal: matmuls N-sliced to q >= k-block start; key-length
     mask is a per-partition bias in the Exp activation; P^T =
     Exp(S^T/8 + bias) written as fp32r, then the diagonal 128-col
     block is multiplied by a 0/1 triangle on gpsimd.
  4. O^T[d, q] (+ row 64 = softmax denominators) = [Vh | 1]^T @ P^T
     accumulated over k-blocks in PSUM. K projections are emitted per
     head pair so they overlap the ACT-bound softmax stream.
  5. recip = qmask / denominators; gpsimd partition-broadcast; DVE
     normalize; O^T DMA'd out (host transposes during reassembly).

No collectives: host shards inputs and reassembles the output.
"""

import sys

if "/opt/trn_rl_repo" not in sys.path:
    sys.path.insert(0, "/opt/trn_rl_repo")

from contextlib import ExitStack

import numpy as np

import concourse.bacc as bacc
import concourse.tile as tile
from concourse import mybir

B, S, DM, H, D = 4, 1024, 1024, 16, 64
NH = 512  # per-core output head-dims (8 heads x 64)
NEG = 1e12
f32, f32r = mybir.dt.float32, mybir.dt.float32r
Exp = mybir.ActivationFunctionType.Exp

_NC = None


def _emit(nc, tc, ctx, reps=1):
    Qd = nc.dram_tensor("QT", [128, 8, S], f32r, kind="ExternalInput")
    Kd = nc.dram_tensor("KT", [128, 8, S], f32r, kind="ExternalInput")
    Vd = nc.dram_tensor("VT", [128, 8, S], f32r, kind="ExternalInput")
    Wqd = nc.dram_tensor("Wq", [128, 8, NH], f32r, kind="ExternalInput")
    Wkd = nc.dram_tensor("Wk", [128, 8, NH], f32r, kind="ExternalInput")
    Wvd = nc.dram_tensor("Wv", [128, 8, NH], f32r, kind="ExternalInput")
    vbd = nc.dram_tensor("vbias", [128, 8], f32, kind="ExternalInput")
    qmd = nc.dram_tensor("qmask", [1, S], f32, kind="ExternalInput")
    Od = nc.dram_tensor("OT", [NH, S], f32, kind="ExternalOutput")

    cons = ctx.enter_context(tc.tile_pool(name="cons", bufs=1))
    xt_pool = ctx.enter_context(tc.tile_pool(name="xt", bufs=2))
    wpool = ctx.enter_context(tc.tile_pool(name="w", bufs=2))
    qk_pool = ctx.enter_context(tc.tile_pool(name="qk", bufs=2))
    vh_pool = ctx.enter_context(tc.tile_pool(name="vh", bufs=1))
    pt_pool = ctx.enter_context(tc.tile_pool(name="pt", bufs=4))
    nrm_pool = ctx.enter_context(tc.tile_pool(name="nrm", bufs=3))
    sm_pool = ctx.enter_context(tc.tile_pool(name="sm", bufs=4))
    ps_mm = ctx.enter_context(tc.tile_pool(name="psmm", bufs=2, space="PSUM"))
    ps_ot = ctx.enter_context(tc.tile_pool(name="psot", bufs=4, space="PSUM"))

    # tri01[p, t] = 1 if t >= p else 0  (zeroes q < k on the diagonal block,
    # applied multiplicatively to P = exp(S) on gpsimd after the exp)
    tri0 = cons.tile([128, 128], f32, tag="tri0")
    nc.vector.memset(tri0, 1.0)
    nc.gpsimd.affine_select(
        out=tri0,
        in_=tri0,
        compare_op=mybir.AluOpType.is_ge,
        fill=0.0,
        base=0,
        pattern=[[1, 128]],
        channel_multiplier=-1,
    )
    tri01 = cons.tile([128, 128], f32r, tag="tri01")
    nc.vector.tensor_copy(tri01, tri0)
    vb = cons.tile([128, 8], f32, tag="vb")
    nc.sync.dma_start(out=vb, in_=vbd.ap())
    qm = cons.tile([1, S], f32, tag="qm")
    nc.sync.dma_start(out=qm, in_=qmd.ap())
    ones0 = cons.tile([128, 1], f32, tag="ones0")
    nc.vector.memset(ones0, 1.0)
    onesf = cons.tile([128, 1], f32r, tag="onesf")
    nc.vector.tensor_copy(onesf, ones0)

    def body(rep):
        def load_xt(dram, eng2):
            # [DM, S] fp32r -> [128, 8 dm-chunks, S], 1MB per DMA, two rings
            xt = xt_pool.tile([128, 8, S], f32r, tag="xt", name="xt")
            view = dram.ap()
            for h in range(4):
                e = nc.sync if h % 2 == 0 else eng2
                e.dma_start(out=xt[:, 2 * h : 2 * h + 2], in_=view[:, 2 * h : 2 * h + 2])
            return xt

        def load_w(dram, eng, split=1):
            w = wpool.tile([128, 8, NH], f32r, tag="w", name="w")
            view = dram.ap()
            step = 8 // split
            for i in range(split):
                eng.dma_start(
                    out=w[:, i * step : (i + 1) * step],
                    in_=view[:, i * step : (i + 1) * step],
                )
            return w

        def project_qk(xt, w):
            out_t = qk_pool.tile([128, 4, S], f32r, tag="qk", name="qk")
            for hp in range(4):
                for qc in range(2):
                    pm = ps_mm.tile([128, 2, 512], f32, tag="mm", name="pm")[:, 0]
                    for c in range(8):
                        nc.tensor.matmul(
                            pm,
                            w[:, c, hp * 128 : (hp + 1) * 128],
                            xt[:, c, qc * 512 : (qc + 1) * 512],
                            start=(c == 0),
                            stop=(c == 7),
                        )
                    dst = out_t[:, hp, qc * 512 : (qc + 1) * 512]
                    if (hp + qc) % 2 == 0:
                        nc.vector.tensor_copy(out=dst, in_=pm)
                    else:
                        nc.scalar.copy(dst, pm)
            return out_t

        wv = load_w(Wvd, nc.sync, split=2)
        xt = load_xt(Vd, nc.gpsimd)
        # V natural [k, hd] + ones column per head: [128k, 8 kc, 8 heads, 65]
        VhO = vh_pool.tile([128, 8, 8, 65], f32r, tag="vh", name="vh")
        nc.vector.tensor_copy(
            out=VhO[:, :, :, 64:65],
            in_=onesf[:, None, None, :].to_broadcast([128, 8, 8, 1]),
        )
        for kc in range(8):
            pm = ps_mm.tile([128, 2, 512], f32, tag="mm", name="pmv")[:, 0]
            for c in range(8):
                nc.tensor.matmul(
                    pm,
                    xt[:, c, kc * 128 : (kc + 1) * 128],
                    wv[:, c],
                    start=(c == 0),
                    stop=(c == 7),
                )
            nc.vector.tensor_copy(
                out=VhO[:, kc, :, 0:64],
                in_=pm.rearrange("p (h d) -> p h d", h=8),
            )
        wq = load_w(Wqd, nc.gpsimd)
        xtq = load_xt(Qd, nc.gpsimd)
        QhT = project_qk(xtq, wq)
        wk = load_w(Wkd, nc.gpsimd)
        xtk = load_xt(Kd, nc.gpsimd)
        KhT = qk_pool.tile([128, 4, S], f32r, tag="qk", name="kht")

        # --- attention (K projection per head pair interleaves with the
        # ACT-bound softmax stream of the previous pair) ---
        for hp in range(4):
            for qc2 in range(2):
                pm = ps_mm.tile([128, 2, 512], f32, tag="mm", name="pmk")[:, 0]
                for c in range(8):
                    nc.tensor.matmul(
                        pm,
                        wk[:, c, hp * 128 : (hp + 1) * 128],
                        xtk[:, c, qc2 * 512 : (qc2 + 1) * 512],
                        start=(c == 0),
                        stop=(c == 7),
                    )
                dst = KhT[:, hp, qc2 * 512 : (qc2 + 1) * 512]
                if qc2 == 0:
                    nc.vector.tensor_copy(out=dst, in_=pm)
                else:
                    nc.scalar.copy(dst, pm)
            for qc in range(2):
                kmax = 4 * (qc + 1)
                ots = [
                    ps_ot.tile([128, 512], f32, tag="ot", name=f"ot{i}")[:65]
                    for i in range(2)
                ]
                for ki in range(kmax):
                    off = max(0, ki * 128 - qc * 512)
                    ksl = slice(ki * 128, (ki + 1) * 128)
                    diag = ki >= qc * 4
                    st = ps_mm.tile([128, 2, 512], f32, tag="mm", name="st")
                    # fp32r runs 4x slower under N=256; widen narrow chunks
                    # (the extra columns are never read downstream)
                    qk_off = min(off, 256)
                    qk_qsl = slice(qc * 512 + qk_off, (qc + 1) * 512)
                    for hx in range(2):
                        hrow = slice(64 * hx, 64 * hx + 64)
                        nc.tensor.matmul(
                            st[:, hx, qk_off:],
                            KhT[hrow, hp, ksl],
                            QhT[hrow, hp, qk_qsl],
                            start=True,
                            stop=True,
                        )
                    pt = pt_pool.tile([128, 2, 512], f32r, tag="pt", name="pt")
                    nc.scalar.activation(
                        pt[:, :, off:], st[:, :, off:], Exp,
                        bias=vb[:, ki : ki + 1], scale=0.125,
                    )
                    if diag:
                        nc.gpsimd.tensor_mul(
                            pt[:, :, off : off + 128],
                            pt[:, :, off : off + 128],
                            tri01[:, None, :].to_broadcast([128, 2, 128]),
                        )
                    for hx in range(2):
                        nc.tensor.matmul(
                            ots[hx][:, off:],
                            VhO[:, ki, 2 * hp + hx, :],
                            pt[:, hx, off:],
                            start=(ki == 0),
                            stop=(ki == kmax - 1),
                        )
                for hx in range(2):
                    ot = ots[hx]
                    recip = sm_pool.tile([1, 512], f32, tag="sm", name="recip")
                    nc.vector.reciprocal(recip, ot[64:65, :])
                    nc.vector.tensor_mul(
                        recip, recip, qm[:, qc * 512 : (qc + 1) * 512]
                    )
                    rbc = nrm_pool.tile([64, 512], f32, tag="rbc", name="rbc")
                    nc.gpsimd.partition_broadcast(rbc, recip)
                    osb = nrm_pool.tile([64, 512], f32, tag="osb", name="osb")
                    nc.vector.tensor_mul(osb, ot[0:64, :], rbc)
                    hrow0 = hp * 128 + hx * 64
                    eng = nc.sync if hx == 0 else nc.scalar
                    eng.dma_start(
                        out=Od.ap()[hrow0 : hrow0 + 64, qc * 512 : (qc + 1) * 512],
                        in_=osb,
                    )

    if reps == 1:
        body(0)
    else:
        for r in range(reps):
            body(r)


def _build(reps=1):
    nc = bacc.Bacc("TRN2", target_bir_lowering=False, debug=False)
    with tile.TileContext(nc) as tc, ExitStack() as ctx:
        _emit(nc, tc, ctx, reps=reps)
    nc.compile()
    return nc


def get_nc(reps=1):
    global _NC
    if reps != 1:
        return _build(reps)
    if _NC is None:
        _NC = _build()
    return _NC


def _round_f32r(a):
    """Round fp32 to fp32r (11-bit mantissa, RNE) — what the PE consumes."""
    b = np.ascontiguousarray(a, np.float32).view(np.uint32)
    keep = b & np.uint32(0xFFFFF000)
    low = b & np.uint32(0xFFF)
    rnd = (low > 0x800) | ((low == 0x800) & (((b >> np.uint32(12)) & 1) == 1))
    out = keep + (rnd.astype(np.uint32) << np.uint32(12))
    return out.view(np.float32)


def make_in_maps(Q_seq, K_seq, V_seq, WQ, WK, WV, Q_len, V_len):
    karange = np.arange(S)
    in_maps = []
    def pack(xt_2d):
        # [DM, n] -> [128, 8, n], row c*128+p -> [p, c]
        return np.ascontiguousarray(
            xt_2d.reshape(8, 128, xt_2d.shape[1]).transpose(1, 0, 2)
        )

    qt = [_round_f32r(pack(Q_seq[b].T)) for b in range(B)]
    kt = [_round_f32r(pack(K_seq[b].T)) for b in range(B)]
    vt = [_round_f32r(pack(V_seq[b].T)) for b in range(B)]
    wq = [_round_f32r(pack(WQ[:, hh * NH : (hh + 1) * NH])) for hh in range(2)]
    wk = [_round_f32r(pack(WK[:, hh * NH : (hh + 1) * NH])) for hh in range(2)]
    wv = [_round_f32r(pack(WV[:, hh * NH : (hh + 1) * NH])) for hh in range(2)]
    for c in range(8):
        b, hh = c // 2, c % 2
        vbias = np.where(karange < int(V_len[b, 0]), 0.0, -NEG).astype(np.float32)
        qmask = (karange < int(Q_len[b, 0])).astype(np.float32)
        in_maps.append(
            {
                "QT": qt[b],
                "KT": kt[b],
                "VT": vt[b],
                "Wq": wq[hh],
                "Wk": wk[hh],
                "Wv": wv[hh],
                "vbias": np.ascontiguousarray(vbias.reshape(8, 128).T),
                "qmask": qmask[None, :],
            }
        )
    return in_maps


def assemble(results):
    out = np.empty((B, S, H * D), np.float32)
    for c in range(8):
        b, hh = c // 2, c % 2
        out[b, :, hh * NH : (hh + 1) * NH] = results[c]["OT"].T
    return out


def kernel(Q_seq, K_seq, V_seq, WQ, WK, WV, Q_len, V_len):
    from concourse.bass_utils import run_bass_kernel_spmd

    nc = get_nc()
    in_maps = make_in_maps(Q_seq, K_seq, V_seq, WQ, WK, WV, Q_len, V_len)
    r = run_bass_kernel_spmd(nc, in_maps, core_ids=list(range(8)))
    return assemble(r.results)

